# revision 1
# baseline (speedup 1.0000x reference)
"""Longformer sliding-chunk attention (B=2, S=4096, E=1024, H=16, W=256) on 8 trn2 cores.

Sharding: tensor-parallel over heads — core c owns heads {2c, 2c+1}. Each core:
  - projects q/k/v for its 128 output features (2 heads x 64) over the full
    [8192, 1024] hidden states, directly in transposed [d, s] layout
  - computes chunked attention fully transposed: scoresT = K @ Q^T per
    128-key-block, exp on ACT (no max subtraction: scores are O(1) for this
    problem), probsT @ V via PE with an appended ones-column that yields the
    softmax denominators for free
  - ships unnormalized numerator^T [128, 8192] + denominators [2, 8192]
Host adds the boundary-mask pad mass to denominators and normalizes.

All matmuls run in float32r (full-rate fp32 mode, ~1e-4 rounding).
"""
import numpy as np

import concourse.bass as bass
import concourse.mybir as mybir
import concourse.tile as tile
from concourse import bacc
from concourse.bass_utils import run_bass_kernel_spmd
from concourse.masks import make_identity

F32 = mybir.dt.float32
F32R = mybir.dt.float32r
AFT = mybir.ActivationFunctionType

B, S, E = 2, 4096, 1024
H, W, D = 16, 256, 64
BS = B * S           # 8192
NT = 16              # 512-wide seq tiles over BS for projections
KT = 8               # contraction tiles of 128 over E
NCHUNK = S // W      # 16 chunks per batch
NKB = S // 128       # 32 key blocks of 128 per batch

_NC_CACHE = None


def _build():
    nc = bacc.Bacc("TRN2", target_bir_lowering=False, debug=False, num_devices=8)

    hsT = nc.dram_tensor("hsT", [E, BS], F32R, kind="ExternalInput").ap()
    w_ap = {}
    b_ap = {}
    for nm in ("q", "k", "v"):
        w_ap[nm] = nc.dram_tensor(f"w{nm}T", [E, 128], F32R, kind="ExternalInput").ap()
        b_ap[nm] = nc.dram_tensor(f"b{nm}", [128, 1], F32, kind="ExternalInput").ap()
    ones2 = nc.dram_tensor("ones2", [128, 24], F32R, kind="ExternalInput").ap()
    outT = nc.dram_tensor("outT", [130, BS], F32, kind="ExternalOutput").ap()

    with tile.TileContext(nc) as tc:
        with (
            tc.tile_pool(name="singles", bufs=1) as singles,
            tc.tile_pool(name="big", bufs=1) as big,
            tc.tile_pool(name="hst", bufs=4) as hpool,
            tc.tile_pool(name="probs", bufs=4) as probs_pool,
            tc.tile_pool(name="vones", bufs=12) as vpool,
            tc.tile_pool(name="stage", bufs=4) as stage_pool,
            tc.tile_pool(name="psmm", bufs=4, space="PSUM") as ps_mm,
            tc.tile_pool(name="pspv", bufs=2, space="PSUM") as ps_pv,
            tc.tile_pool(name="psvt", bufs=2, space="PSUM") as ps_vt,
        ):
            ident = singles.tile([128, 128], F32)
            make_identity(nc, ident)

            w_sb = {}
            b_sb = {}
            for nm in ("q", "k", "v"):
                wt = singles.tile([128, KT, 128], F32R, tag=f"w{nm}")
                nc.sync.dma_start(
                    out=wt, in_=w_ap[nm].rearrange("(kt p) m -> p kt m", p=128)
                )
                w_sb[nm] = wt
                bt = singles.tile([128, 1], F32, tag=f"b{nm}")
                nc.sync.dma_start(out=bt, in_=b_ap[nm])
                b_sb[nm] = bt

            QT = big.tile([128, BS], F32R, tag="qt")
            vring = big.tile([128, 12, 130], F32R, tag="vring")
            nc.sync.dma_start(
                out=vring.rearrange("p s (x o) -> p s x o", x=2)[:, :, :, 64:65],
                in_=ones2.rearrange("p (s x o) -> p s x o", s=12, x=2, o=1),
            )
            KTt = big.tile([128, BS], F32R, tag="kt")
            VT = big.tile([128, BS], F32R, tag="vt")

            # ---- Phase 1: projections, output in [feature, seq] layout ----
            hsT_r = hsT.rearrange("(kt p) s -> p kt s", p=128)
            for n in range(NT):
                sl = slice(n * 512, (n + 1) * 512)
                hst0 = hpool.tile([128, 4, 512], F32R, tag="hst")
                hst1 = hpool.tile([128, 4, 512], F32R, tag="hst")
                nc.sync.dma_start(out=hst0, in_=hsT_r[:, 0:4, sl])
                nc.sync.dma_start(out=hst1, in_=hsT_r[:, 4:8, sl])
                halves = (hst0, hst1)
                for nm, dest, scale in (
                    ("q", QT, 1.0 / np.sqrt(D)),
                    ("k", KTt, 1.0),
                    ("v", VT, 1.0),
                ):
                    psp = ps_mm.tile([128, 512], F32, tag="mm")
                    for k in range(KT):
                        nc.tensor.matmul(
                            psp,
                            lhsT=w_sb[nm][:, k, :],
                            rhs=halves[k // 4][:, k % 4, :],
                            start=(k == 0),
                            stop=(k == KT - 1),
                        )
                    nc.scalar.activation(
                        dest[:, sl], psp, AFT.Identity, bias=b_sb[nm], scale=scale
                    )

            # ---- Phase 2: chunked attention, fully transposed ----
            vones = {}
            for b in range(B):
                base = b * S
                for c in range(NCHUNK):
                    lo = max(0, 2 * c - 2)
                    hi = min(NKB, 2 * c + 4)
                    n_kb = hi - lo

                    # V^T -> [keys, d] ring slots (+persistent ones col)
                    for kb in range(lo, hi):
                        if (b, kb) in vones:
                            continue
                        vt_ps = ps_vt.tile([128, 128], F32, tag="vt")
                        nc.tensor.transpose(
                            vt_ps,
                            VT[:, base + kb * 128 : base + (kb + 1) * 128].bitcast(F32),
                            ident,
                        )
                        slot = (2 * NKB * b + kb) % 12
                        nc.vector.tensor_copy(
                            vring[:, slot, :].rearrange("p (h x) -> p h x", h=2)[
                                :, :, 0:64
                            ],
                            vt_ps.rearrange("p (h x) -> p h x", h=2),
                        )
                        vones[(b, kb)] = slot

                    q_sl = slice(base + c * W, base + (c + 1) * W)
                    pr = {
                        h: probs_pool.tile(
                            [128, 6, 256], F32R, tag="probs", name=f"pr{h}_{b}_{c}"
                        )
                        for h in (0, 1)
                    }
                    for ip in range(n_kb // 2):
                        sps = {
                            h: ps_mm.tile(
                                [128, 2, 256], F32, tag="mm", name=f"s{h}_{b}_{c}_{ip}"
                            )
                            for h in (0, 1)
                        }
                        for j in (0, 1):
                            kb = lo + 2 * ip + j
                            k_sl = slice(base + kb * 128, base + (kb + 1) * 128)
                            for h in (0, 1):
                                d_sl = slice(h * 64, (h + 1) * 64)
                                nc.tensor.matmul(
                                    sps[h][:, j, :],
                                    lhsT=KTt[d_sl, k_sl],
                                    rhs=QT[d_sl, q_sl],
                                    start=True,
                                    stop=True,
                                )
                        for h in (0, 1):
                            nc.scalar.activation(
                                pr[h][:, 2 * ip : 2 * ip + 2, :], sps[h], AFT.Exp
                            )

                    o_sl_pre = slice(base + c * W, base + (c + 1) * W)
                    stage = stage_pool.tile([128, 256], F32, tag="stage")
                    for h in (0, 1):
                        po = ps_pv.tile([65, 256], F32, tag="pv")
                        for i in range(n_kb):
                            kb = lo + i
                            nc.tensor.matmul(
                                po,
                                lhsT=vring[:, vones[(b, kb)], h * 65 : (h + 1) * 65],
                                rhs=pr[h][:, i, :],
                                start=(i == 0),
                                stop=(i == n_kb - 1),
                            )
                        nc.vector.tensor_copy(stage[h * 64 : (h + 1) * 64, :], po[0:64, :])
                        dst_h = stage_pool.tile(
                            [1, 256], F32, tag=f"dstage{h}", name=f"dst{h}_{b}_{c}"
                        )
                        nc.vector.tensor_copy(dst_h, po[64:65, :])
                        nc.sync.dma_start(
                            out=outT[128 + h : 129 + h, o_sl_pre], in_=dst_h
                        )

                    nc.sync.dma_start(out=outT[0:128, o_sl_pre], in_=stage)

    nc.compile()
    return nc


def get_nc():
    global _NC_CACHE
    if _NC_CACHE is None:
        _NC_CACHE = _build()
    return _NC_CACHE


def make_in_maps(hidden_states, Wq, bq, Wk, bk, Wv, bv):
    hsT = np.ascontiguousarray(
        hidden_states.reshape(BS, E).T.astype(np.float32, copy=False)
    )
    ones2 = np.ones((128, 24), np.float32)
    in_maps = []
    for c in range(8):
        fsl = slice(c * 128, (c + 1) * 128)
        in_maps.append(
            {
                "hsT": hsT,
                "wqT": np.ascontiguousarray(Wq[fsl].T.astype(np.float32, copy=False)),
                "wkT": np.ascontiguousarray(Wk[fsl].T.astype(np.float32, copy=False)),
                "wvT": np.ascontiguousarray(Wv[fsl].T.astype(np.float32, copy=False)),
                "bq": np.ascontiguousarray(bq[fsl].reshape(128, 1) / np.sqrt(D)),
                "bk": np.ascontiguousarray(bk[fsl].reshape(128, 1)),
                "bv": np.ascontiguousarray(bv[fsl].reshape(128, 1)),
                "ones2": ones2,
            }
        )
    return in_maps


def assemble(results):
    """results: list of 8 per-core dicts with 'outT' [130, BS] -> full [B,S,E]."""
    # boundary pad mass: chunk 0 row ii has ii unmasked zero-score pad keys,
    # chunk 15 row ii has 255-ii
    pad = np.zeros(S, np.float32)
    pad[:W] = np.arange(W, dtype=np.float32)
    pad[S - W :] = (W - 1) - np.arange(W, dtype=np.float32)

    out = np.empty((B, S, E), np.float32)
    for c in range(8):
        oT = results[c]["outT"]  # [130, BS]
        num = oT[0:128].T.reshape(B, S, 2, 64)  # b, s, head_local, d
        den = oT[128:130].T.reshape(B, S, 2)  # b, s, head_local
        den = den + pad[None, :, None]
        out[:, :, c * 128 : (c + 1) * 128] = (num / den[..., None]).reshape(B, S, 128)
    return out


def kernel(hidden_states, Wq, bq, Wk, bk, Wv, bv):
    nc = get_nc()
    in_maps = make_in_maps(hidden_states, Wq, bq, Wk, bk, Wv, bv)
    res = run_bass_kernel_spmd(nc, in_maps, list(range(8)))
    return assemble(res.results)



# revision 5
# speedup vs baseline: 291.0474x; 291.0474x over previous
"""Longformer sliding-chunk attention (B=2, S=4096, E=1024, H=16, W=256) on 8 trn2 cores.

Sharding: tensor-parallel over heads — core c owns heads {2c, 2c+1}. Each core:
  - projects q/k/v for its 128 output features (2 heads x 64) over the full
    [8192, 1024] hidden states, in transposed [d, s] layout, bf16
  - computes chunked attention fully transposed and software-pipelined with the
    projections (per 512-seq group: project, transpose new V blocks, compute
    scoresT = K @ Q^T per 128-key-block over its full 768-query window, exp on
    ACT into a bf16 probs ring, then probsT @ V for completed chunks with an
    appended ones-column yielding softmax denominators for free)
  - ships unnormalized numerator^T [128, 8192] + denominators [2, 8192]
Host adds the boundary-mask pad mass to denominators and normalizes.

All matmuls run in bf16 (fp32 PSUM accumulation). No max-subtraction before
exp: scores are O(1) for this problem.
"""
import numpy as np
import ml_dtypes

import concourse.bass as bass
import concourse.mybir as mybir
import concourse.tile as tile
from concourse import bacc
from concourse.bass_utils import run_bass_kernel_spmd
from concourse.masks import make_identity

F32 = mybir.dt.float32
BF16 = mybir.dt.bfloat16
AFT = mybir.ActivationFunctionType

B, S, E = 2, 4096, 1024
H, W, D = 16, 256, 64
BS = B * S           # 8192
KT = 8               # contraction tiles of 128 over E
NCHUNK = S // W      # 16 chunks per batch
NKB = S // 128       # 32 key blocks of 128 per batch
NG = 8               # 512-seq groups per batch
PR = 10              # probs ring slots per head
VR = 16              # vring slots

_NC_CACHE = None


def _score_window(kb):
    """Chunk range [w0, w1) of queries attending key block kb."""
    w0 = max(0, kb // 2 - 1)
    w1 = min(NCHUNK, kb // 2 + 2)
    return w0, w1


def _scores_ready(kb, g):
    """Can scores for key block kb be emitted after local group g of its batch?"""
    have = (g + 1) * 512
    if (kb + 1) * 128 > have:
        return False
    _, w1 = _score_window(kb)
    return w1 * 256 <= have


def _build():
    nc = bacc.Bacc("TRN2", target_bir_lowering=False, debug=False, num_devices=8)

    hsT = nc.dram_tensor("hsT", [E, BS], BF16, kind="ExternalInput").ap()
    w_ap = {}
    b_ap = {}
    for nm in ("q", "k", "v"):
        w_ap[nm] = nc.dram_tensor(f"w{nm}T", [E, 128], BF16, kind="ExternalInput").ap()
        b_ap[nm] = nc.dram_tensor(f"b{nm}", [128, 1], F32, kind="ExternalInput").ap()
    onesd = nc.dram_tensor("onesd", [128, 2 * VR], BF16, kind="ExternalInput").ap()
    outT = nc.dram_tensor("outT", [130, BS], F32, kind="ExternalOutput").ap()

    with tile.TileContext(nc) as tc:
        with (
            tc.tile_pool(name="singles", bufs=1) as singles,
            tc.tile_pool(name="big", bufs=1) as big,
            tc.tile_pool(name="hst", bufs=3) as hpool,
            tc.tile_pool(name="stage", bufs=4) as stage_pool,
            tc.tile_pool(name="den", bufs=4) as den_pool,
            tc.tile_pool(name="psB", bufs=2, space="PSUM") as psB,   # proj + scores
            tc.tile_pool(name="psC", bufs=3, space="PSUM") as psC,   # pv accumulators
            tc.tile_pool(name="psD", bufs=1, space="PSUM") as psD,   # v transposes
        ):
            ident = singles.tile([128, 128], BF16)
            make_identity(nc, ident)

            w_sb = {}
            b_sb = {}
            for nm in ("q", "k", "v"):
                wt = singles.tile([128, KT, 128], BF16, tag=f"w{nm}")
                nc.sync.dma_start(
                    out=wt, in_=w_ap[nm].rearrange("(kt p) m -> p kt m", p=128)
                )
                w_sb[nm] = wt
                bt = singles.tile([128, 1], F32, tag=f"b{nm}")
                nc.sync.dma_start(out=bt, in_=b_ap[nm])
                b_sb[nm] = bt

            QT = big.tile([128, BS], BF16, tag="qt")
            KTt = big.tile([128, BS], BF16, tag="kt")
            VT = big.tile([128, BS], BF16, tag="vt")
            vring = big.tile([128, VR, 130], BF16, tag="vring")
            nc.sync.dma_start(
                out=vring.rearrange("p s (x o) -> p s x o", x=2)[:, :, :, 64:65],
                in_=onesd.rearrange("p (s x o) -> p s x o", s=VR, x=2, o=1),
            )
            probs = {
                h: big.tile([128, PR, 768], BF16, tag=f"probs{h}", name=f"probs{h}")
                for h in (0, 1)
            }

            hsT_r = hsT.rearrange("(kt p) s -> p kt s", p=128)

            def emit_transpose(b, kb):
                base = b * S
                vt = psD.tile([128, 128], BF16, tag="vt")
                nc.tensor.transpose(
                    vt, VT[:, base + kb * 128 : base + (kb + 1) * 128], ident
                )
                slot = (b * NKB + kb) % VR
                nc.vector.tensor_copy(
                    vring[:, slot, :].rearrange("p (h x) -> p h x", h=2)[:, :, 0:64],
                    vt.rearrange("p (h x) -> p h x", h=2),
                )

            def emit_scores(b, kb, h):
                base = b * S
                w0, w1 = _score_window(kb)
                q0 = base + w0 * 256
                width = (w1 - w0) * 256
                d_sl = slice(h * 64, (h + 1) * 64)
                k_sl = slice(base + kb * 128, base + (kb + 1) * 128)
                sp = psB.tile([128, 1024], F32, tag="mm")
                nc.tensor.matmul(
                    sp[:, 0:512],
                    lhsT=KTt[d_sl, k_sl],
                    rhs=QT[d_sl, q0 : q0 + 512],
                    start=True, stop=True,
                )
                if width > 512:
                    nc.tensor.matmul(
                        sp[:, 512:768],
                        lhsT=KTt[d_sl, k_sl],
                        rhs=QT[d_sl, q0 + 512 : q0 + 768],
                        start=True, stop=True,
                    )
                slot = (b * NKB + kb) % PR
                nc.scalar.activation(
                    probs[h][:, slot, 0:width], sp[:, 0:width], AFT.Exp
                )

            def emit_chunk(b, c):
                base = b * S
                lo = max(0, 2 * c - 2)
                hi = min(NKB, 2 * c + 4)
                o_sl = slice(base + c * W, base + (c + 1) * W)
                stage = stage_pool.tile([128, 256], F32, tag="stage")
                for h in (0, 1):
                    po = psC.tile([65, 256], F32, tag="pv")
                    for i, kb in enumerate(range(lo, hi)):
                        w0, _ = _score_window(kb)
                        slot = (b * NKB + kb) % PR
                        off = (c - w0) * 256
                        nc.tensor.matmul(
                            po,
                            lhsT=vring[
                                :, (b * NKB + kb) % VR, h * 65 : (h + 1) * 65
                            ],
                            rhs=probs[h][:, slot, off : off + 256],
                            start=(i == 0),
                            stop=(i == hi - lo - 1),
                        )
                    nc.vector.tensor_copy(
                        stage[h * 64 : (h + 1) * 64, :], po[0:64, :]
                    )
                    den_h = den_pool.tile(
                        [1, 256], F32, tag=f"den{h}", name=f"den{h}_{b}_{c}"
                    )
                    nc.vector.tensor_copy(den_h, po[64:65, :])
                    nc.sync.dma_start(out=outT[128 + h : 129 + h, o_sl], in_=den_h)
                nc.sync.dma_start(out=outT[0:128, o_sl], in_=stage)

            # Software pipeline: phase-2 work that becomes ready after local
            # group lg is emitted one global group later, so the PE never
            # waits on the just-written Q/K/V of the current group.
            state = [{"kb": 0, "c": 0} for _ in range(B)]
            for gg in range(B * NG + 1):
                if gg < B * NG:
                    b, g = divmod(gg, NG)
                    gsl = slice(gg * 512, (gg + 1) * 512)
                    gt = hpool.tile([128, KT, 512], BF16, tag="hst")
                    if gg == 0:
                        for kt in range(KT):
                            nc.sync.dma_start(out=gt[:, kt, :], in_=hsT_r[:, kt, gsl])
                    else:
                        nc.sync.dma_start(out=gt, in_=hsT_r[:, :, gsl])
                    for nm, dest, scale in (
                        ("q", QT, 1.0 / np.sqrt(D)),
                        ("k", KTt, 1.0),
                        ("v", VT, 1.0),
                    ):
                        ps = psB.tile([128, 1024], F32, tag="mm")
                        for kt in range(KT):
                            nc.tensor.matmul(
                                ps[:, 0:512],
                                lhsT=w_sb[nm][:, kt, :],
                                rhs=gt[:, kt, :],
                                start=(kt == 0),
                                stop=(kt == KT - 1),
                            )
                        nc.scalar.activation(
                            dest[:, gsl], ps[:, 0:512], AFT.Identity,
                            bias=b_sb[nm], scale=scale,
                        )

                pg = gg - 1
                if pg < 0:
                    continue
                b2, lg = divmod(pg, NG)
                st = state[b2]
                t_kbs = list(range(4 * lg, min(4 * lg + 4, NKB)))
                s_kbs = []
                while st["kb"] < NKB and _scores_ready(st["kb"], lg):
                    s_kbs.append(st["kb"])
                    st["kb"] += 1
                c_done = []
                while st["c"] < NCHUNK and min(NKB, 2 * st["c"] + 4) <= st["kb"]:
                    c_done.append(st["c"])
                    st["c"] += 1

                ti = 0
                for kb in s_kbs:
                    if ti < len(t_kbs):
                        emit_transpose(b2, t_kbs[ti])
                        ti += 1
                    for h in (0, 1):
                        emit_scores(b2, kb, h)
                while ti < len(t_kbs):
                    emit_transpose(b2, t_kbs[ti])
                    ti += 1
                for c in c_done:
                    emit_chunk(b2, c)

    nc.compile()
    return nc


def get_nc():
    global _NC_CACHE
    if _NC_CACHE is None:
        _NC_CACHE = _build()
    return _NC_CACHE


def make_in_maps(hidden_states, Wq, bq, Wk, bk, Wv, bv):
    bf16 = ml_dtypes.bfloat16
    hsT = np.ascontiguousarray(
        hidden_states.reshape(BS, E).T.astype(np.float32, copy=False)
    ).astype(bf16)
    onesd = np.ones((128, 2 * VR), bf16)
    in_maps = []
    for c in range(8):
        fsl = slice(c * 128, (c + 1) * 128)
        in_maps.append(
            {
                "hsT": hsT,
                "wqT": np.ascontiguousarray(Wq[fsl].T.astype(np.float32)).astype(bf16),
                "wkT": np.ascontiguousarray(Wk[fsl].T.astype(np.float32)).astype(bf16),
                "wvT": np.ascontiguousarray(Wv[fsl].T.astype(np.float32)).astype(bf16),
                "bq": np.ascontiguousarray(
                    bq[fsl].reshape(128, 1).astype(np.float32) / np.sqrt(D)
                ),
                "bk": np.ascontiguousarray(bk[fsl].reshape(128, 1).astype(np.float32)),
                "bv": np.ascontiguousarray(bv[fsl].reshape(128, 1).astype(np.float32)),
                "onesd": onesd,
            }
        )
    return in_maps


def assemble(results):
    """results: list of 8 per-core dicts with 'outT' [130, BS] -> full [B,S,E]."""
    # boundary pad mass: chunk 0 row ii has ii unmasked zero-score pad keys,
    # chunk 15 row ii has 255-ii
    pad = np.zeros(S, np.float32)
    pad[:W] = np.arange(W, dtype=np.float32)
    pad[S - W :] = (W - 1) - np.arange(W, dtype=np.float32)

    out = np.empty((B, S, E), np.float32)
    for c in range(8):
        oT = results[c]["outT"]  # [130, BS]
        num = oT[0:128].T.reshape(B, S, 2, 64)  # b, s, head_local, d
        den = oT[128:130].T.reshape(B, S, 2)  # b, s, head_local
        den = den + pad[None, :, None]
        out[:, :, c * 128 : (c + 1) * 128] = (num / den[..., None]).reshape(B, S, 128)
    return out


def kernel(hidden_states, Wq, bq, Wk, bk, Wv, bv):
    nc = get_nc()
    in_maps = make_in_maps(hidden_states, Wq, bq, Wk, bk, Wv, bv)
    res = run_bass_kernel_spmd(nc, in_maps, list(range(8)))
    return assemble(res.results)


# revision 9
# speedup vs baseline: 293.8091x; 1.0095x over previous
"""Longformer sliding-chunk attention (B=2, S=4096, E=1024, H=16, W=256) on 8 trn2 cores.

Sharding: tensor-parallel over heads — core c owns heads {2c, 2c+1}. Each core:
  - projects q/k/v for its 128 output features (2 heads x 64) over the full
    [8192, 1024] hidden states, in transposed [d, s] layout, bf16
  - computes chunked attention fully transposed and software-pipelined with the
    projections (per 512-seq group: project, transpose new V blocks, compute
    scoresT = K @ Q^T per 128-key-block over its full 768-query window, exp on
    ACT into a bf16 probs ring, then probsT @ V for completed chunks with an
    appended ones-column yielding softmax denominators for free)
  - ships unnormalized numerator^T [128, 8192] + denominators [2, 8192]
Host adds the boundary-mask pad mass to denominators and normalizes.

All matmuls run in bf16 (fp32 PSUM accumulation). No max-subtraction before
exp: scores are O(1) for this problem.
"""
import numpy as np
import ml_dtypes

import concourse.bass as bass
import concourse.mybir as mybir
import concourse.tile as tile
from concourse import bacc
from concourse.bass_utils import run_bass_kernel_spmd
from concourse.masks import make_identity

F32 = mybir.dt.float32
BF16 = mybir.dt.bfloat16
AFT = mybir.ActivationFunctionType

B, S, E = 2, 4096, 1024
H, W, D = 16, 256, 64
BS = B * S           # 8192
KT = 8               # contraction tiles of 128 over E
NCHUNK = S // W      # 16 chunks per batch
NKB = S // 128       # 32 key blocks of 128 per batch
NG = 8               # 512-seq groups per batch
PR = 10              # probs ring slots per head
VR = 16              # vring slots

_NC_CACHE = None


def _score_window(kb):
    """Chunk range [w0, w1) of queries attending key block kb."""
    w0 = max(0, kb // 2 - 1)
    w1 = min(NCHUNK, kb // 2 + 2)
    return w0, w1


def _scores_ready(kb, g):
    """Can scores for key block kb be emitted after local group g of its batch?"""
    have = (g + 1) * 512
    if (kb + 1) * 128 > have:
        return False
    _, w1 = _score_window(kb)
    return w1 * 256 <= have


def _build():
    nc = bacc.Bacc("TRN2", target_bir_lowering=False, debug=False, num_devices=8)

    hsT = nc.dram_tensor("hsT", [E, BS], BF16, kind="ExternalInput").ap()
    # q/k/v weights packed [proj, E, 128] and biases [128, 3] for single DMAs
    wAll = nc.dram_tensor("wAll", [3 * E, 128], BF16, kind="ExternalInput").ap()
    bAll = nc.dram_tensor("bAll", [128, 3], F32, kind="ExternalInput").ap()
    onesd = nc.dram_tensor("onesd", [128, 2 * VR], BF16, kind="ExternalInput").ap()
    outT = nc.dram_tensor("outT", [130, BS], F32, kind="ExternalOutput").ap()

    with tile.TileContext(nc) as tc:
        with (
            tc.tile_pool(name="singles", bufs=1) as singles,
            tc.tile_pool(name="big", bufs=1) as big,
            tc.tile_pool(name="hst", bufs=3) as hpool,
            tc.tile_pool(name="stage", bufs=4) as stage_pool,
            tc.tile_pool(name="den", bufs=4) as den_pool,
            tc.tile_pool(name="psB", bufs=2, space="PSUM") as psB,   # proj + scores
            tc.tile_pool(name="psC", bufs=3, space="PSUM") as psC,   # pv accumulators
            tc.tile_pool(name="psD", bufs=1, space="PSUM") as psD,   # v transposes
        ):
            hsT_r = hsT.rearrange("(kt p) s -> p kt s", p=128)

            # weights/biases first (single packed DMAs on SyncE), first input
            # group split per k-tile on GpSimd so the first matmul can start
            # as early as possible
            w_all = singles.tile([128, 3, KT, 128], BF16, tag="wall")
            nc.sync.dma_start(
                out=w_all, in_=wAll.rearrange("(w kt p) m -> p w kt m", p=128, w=3)
            )
            b_all = singles.tile([128, 3], F32, tag="ball")
            nc.sync.dma_start(out=b_all, in_=bAll)
            w_sb = {nm: w_all[:, i] for i, nm in enumerate(("q", "k", "v"))}
            b_sb = {nm: b_all[:, i : i + 1] for i, nm in enumerate(("q", "k", "v"))}

            gt0 = hpool.tile([128, KT, 512], BF16, tag="hst", name="hst_g0")
            for kt in range(KT):
                nc.gpsimd.dma_start(out=gt0[:, kt, :], in_=hsT_r[:, kt, 0:512])

            QT = big.tile([128, BS], BF16, tag="qt")
            KTt = big.tile([128, BS], BF16, tag="kt")
            VT = big.tile([128, BS], BF16, tag="vt")
            vring = big.tile([128, VR, 130], BF16, tag="vring")
            nc.sync.dma_start(
                out=vring.rearrange("p s (x o) -> p s x o", x=2)[:, :, :, 64:65],
                in_=onesd.rearrange("p (s x o) -> p s x o", s=VR, x=2, o=1),
            )
            probs = {
                h: big.tile([128, PR, 768], BF16, tag=f"probs{h}", name=f"probs{h}")
                for h in (0, 1)
            }

            ident = singles.tile([128, 128], BF16)
            make_identity(nc, ident)

            def emit_transpose(b, kb):
                base = b * S
                vt = psD.tile([128, 128], BF16, tag="vt")
                nc.tensor.transpose(
                    vt, VT[:, base + kb * 128 : base + (kb + 1) * 128], ident
                )
                slot = (b * NKB + kb) % VR
                nc.vector.tensor_copy(
                    vring[:, slot, :].rearrange("p (h x) -> p h x", h=2)[:, :, 0:64],
                    vt.rearrange("p (h x) -> p h x", h=2),
                )

            def emit_scores(b, kb, h):
                base = b * S
                w0, w1 = _score_window(kb)
                q0 = base + w0 * 256
                width = (w1 - w0) * 256
                d_sl = slice(h * 64, (h + 1) * 64)
                k_sl = slice(base + kb * 128, base + (kb + 1) * 128)
                sp = psB.tile([128, 1024], F32, tag="mm")
                nc.tensor.matmul(
                    sp[:, 0:512],
                    lhsT=KTt[d_sl, k_sl],
                    rhs=QT[d_sl, q0 : q0 + 512],
                    start=True, stop=True,
                )
                if width > 512:
                    nc.tensor.matmul(
                        sp[:, 512:768],
                        lhsT=KTt[d_sl, k_sl],
                        rhs=QT[d_sl, q0 + 512 : q0 + 768],
                        start=True, stop=True,
                    )
                slot = (b * NKB + kb) % PR
                nc.scalar.activation(
                    probs[h][:, slot, 0:width], sp[:, 0:width], AFT.Exp
                )

            def emit_chunk(b, c):
                base = b * S
                lo = max(0, 2 * c - 2)
                hi = min(NKB, 2 * c + 4)
                o_sl = slice(base + c * W, base + (c + 1) * W)
                stage = stage_pool.tile([128, 256], F32, tag="stage")
                for h in (0, 1):
                    po = psC.tile([65, 256], F32, tag="pv")
                    for i, kb in enumerate(range(lo, hi)):
                        w0, _ = _score_window(kb)
                        slot = (b * NKB + kb) % PR
                        off = (c - w0) * 256
                        nc.tensor.matmul(
                            po,
                            lhsT=vring[
                                :, (b * NKB + kb) % VR, h * 65 : (h + 1) * 65
                            ],
                            rhs=probs[h][:, slot, off : off + 256],
                            start=(i == 0),
                            stop=(i == hi - lo - 1),
                        )
                    nc.vector.tensor_copy(
                        stage[h * 64 : (h + 1) * 64, :], po[0:64, :]
                    )
                    den_h = den_pool.tile(
                        [1, 256], F32, tag=f"den{h}", name=f"den{h}_{b}_{c}"
                    )
                    nc.vector.tensor_copy(den_h, po[64:65, :])
                    nc.sync.dma_start(out=outT[128 + h : 129 + h, o_sl], in_=den_h)
                nc.sync.dma_start(out=outT[0:128, o_sl], in_=stage)

            # Software pipeline: phase-2 work that becomes ready after local
            # group lg is emitted one global group later, so the PE never
            # waits on the just-written Q/K/V of the current group.
            state = [{"kb": 0, "c": 0} for _ in range(B)]
            for gg in range(B * NG + 1):
                if gg < B * NG:
                    b, g = divmod(gg, NG)
                    gsl = slice(gg * 512, (gg + 1) * 512)
                    if gg == 0:
                        gt = gt0
                    else:
                        gt = hpool.tile([128, KT, 512], BF16, tag="hst")
                        nc.gpsimd.dma_start(out=gt, in_=hsT_r[:, :, gsl])
                    for nm, dest, scale in (
                        ("q", QT, 1.0 / np.sqrt(D)),
                        ("k", KTt, 1.0),
                        ("v", VT, 1.0),
                    ):
                        ps = psB.tile([128, 1024], F32, tag="mm")
                        for kt in range(KT):
                            nc.tensor.matmul(
                                ps[:, 0:512],
                                lhsT=w_sb[nm][:, kt, :],
                                rhs=gt[:, kt, :],
                                start=(kt == 0),
                                stop=(kt == KT - 1),
                            )
                        nc.scalar.activation(
                            dest[:, gsl], ps[:, 0:512], AFT.Identity,
                            bias=b_sb[nm], scale=scale,
                        )

                pg = gg - 1
                if pg < 0:
                    continue
                b2, lg = divmod(pg, NG)
                st = state[b2]
                t_kbs = list(range(4 * lg, min(4 * lg + 4, NKB)))
                s_kbs = []
                while st["kb"] < NKB and _scores_ready(st["kb"], lg):
                    s_kbs.append(st["kb"])
                    st["kb"] += 1
                c_done = []
                while st["c"] < NCHUNK and min(NKB, 2 * st["c"] + 4) <= st["kb"]:
                    c_done.append(st["c"])
                    st["c"] += 1

                ti = 0
                for kb in s_kbs:
                    if ti < len(t_kbs):
                        emit_transpose(b2, t_kbs[ti])
                        ti += 1
                    for h in (0, 1):
                        emit_scores(b2, kb, h)
                while ti < len(t_kbs):
                    emit_transpose(b2, t_kbs[ti])
                    ti += 1
                for c in c_done:
                    emit_chunk(b2, c)

    nc.compile()
    return nc


def get_nc():
    global _NC_CACHE
    if _NC_CACHE is None:
        _NC_CACHE = _build()
    return _NC_CACHE


def make_in_maps(hidden_states, Wq, bq, Wk, bk, Wv, bv):
    bf16 = ml_dtypes.bfloat16
    hsT = np.ascontiguousarray(
        hidden_states.reshape(BS, E).T.astype(np.float32, copy=False)
    ).astype(bf16)
    onesd = np.ones((128, 2 * VR), bf16)
    in_maps = []
    for c in range(8):
        fsl = slice(c * 128, (c + 1) * 128)
        wAll = np.concatenate(
            [np.ascontiguousarray(Wm[fsl].T.astype(np.float32)) for Wm in (Wq, Wk, Wv)],
            axis=0,
        ).astype(bf16)
        bAll = np.stack(
            [
                bq[fsl].astype(np.float32) / np.sqrt(D),
                bk[fsl].astype(np.float32),
                bv[fsl].astype(np.float32),
            ],
            axis=1,
        )
        in_maps.append(
            {
                "hsT": hsT,
                "wAll": np.ascontiguousarray(wAll),
                "bAll": np.ascontiguousarray(bAll),
                "onesd": onesd,
            }
        )
    return in_maps


def assemble(results):
    """results: list of 8 per-core dicts with 'outT' [130, BS] -> full [B,S,E]."""
    # boundary pad mass: chunk 0 row ii has ii unmasked zero-score pad keys,
    # chunk 15 row ii has 255-ii
    pad = np.zeros(S, np.float32)
    pad[:W] = np.arange(W, dtype=np.float32)
    pad[S - W :] = (W - 1) - np.arange(W, dtype=np.float32)

    out = np.empty((B, S, E), np.float32)
    for c in range(8):
        oT = results[c]["outT"]  # [130, BS]
        num = oT[0:128].T.reshape(B, S, 2, 64)  # b, s, head_local, d
        den = oT[128:130].T.reshape(B, S, 2)  # b, s, head_local
        den = den + pad[None, :, None]
        out[:, :, c * 128 : (c + 1) * 128] = (num / den[..., None]).reshape(B, S, 128)
    return out


def kernel(hidden_states, Wq, bq, Wk, bk, Wv, bv):
    nc = get_nc()
    in_maps = make_in_maps(hidden_states, Wq, bq, Wk, bk, Wv, bv)
    res = run_bass_kernel_spmd(nc, in_maps, list(range(8)))
    return assemble(res.results)
